# revision 1
# baseline (speedup 1.0000x reference)
import numpy as np
import jax
import jax.numpy as jnp

# nn_Encoder_Block: B=16,N=128,C=128,H=8,D=16,FF=512; 8 cores, data-parallel on b.
B, N, C, H = 16, 128, 128, 8
D = C // H
FF = 4 * C
SCALE = float(C) ** 0.25
EPS = 1e-5
NCORES = 8

jax.config.update("jax_default_matmul_precision", "highest")


def _ln(x, g, b):
    mu = jnp.mean(x, axis=-1, keepdims=True)
    var = jnp.mean(jnp.square(x - mu), axis=-1, keepdims=True)
    return (x - mu) * jax.lax.rsqrt(var + EPS) * g + b


def _block(x, y, qw, qb, kw, kb, vw, vb, onw, onb, oew, oeb,
           ln1g, ln1b, ln2g, ln2b, ln3g, ln3b, ln4g, ln4b,
           ln5g, ln5b, ln6g, ln6b,
           m1w1, m1b1, m1w2, m1b2, m2w1, m2b1, m2w2, m2b2):
    b, n, c = x.shape
    h = H
    d = C // H
    x1 = _ln(x, ln1g, ln1b)
    y1 = _ln(y, ln2g, ln2b)
    q = (x1 @ qw + qb).reshape(b, h, n, d)
    k = (y1 @ kw + kb).reshape(b, h, n, n, d)
    v = (x1 @ vw + vb).reshape(b, h, n, d)
    out_scores = jnp.einsum('bhmd,bhmnd->bhmn', q, k) * SCALE
    in_scores = jnp.einsum('bhmd,bhmnd->bhnm', q, k) * SCALE
    out_attn = jax.nn.softmax(out_scores, axis=-1)
    in_attn = jax.nn.softmax(in_scores, axis=-1)
    eye = jnp.eye(n, dtype=x.dtype)
    message = out_attn + in_attn - out_attn * eye
    node_h = jnp.einsum('bhmn,bhnd->bhmd', message, v).reshape(b, n, c)
    edge_h = jnp.einsum('bhmn,bhand->bhamd', message, k).reshape(b, n, n, c)
    x2 = _ln(x1 + (node_h @ onw + onb), ln3g, ln3b)
    y2 = _ln(y1 + (edge_h @ oew + oeb), ln4g, ln4b)
    mx = jax.nn.sigmoid(x2 @ m1w1 + m1b1) @ m1w2 + m1b2
    my = jax.nn.sigmoid(y2 @ m2w1 + m2b1) @ m2w2 + m2b2
    x_out = _ln(x2 + mx, ln5g, ln5b)
    y_out = _ln(y2 + my, ln6g, ln6b)
    return x_out, y_out, message


_PARAM_NAMES = ["qw", "qb", "kw", "kb", "vw", "vb", "onw", "onb", "oew", "oeb",
                "ln1g", "ln1b", "ln2g", "ln2b", "ln3g", "ln3b", "ln4g", "ln4b",
                "ln5g", "ln5b", "ln6g", "ln6b",
                "m1w1", "m1b1", "m1w2", "m1b2", "m2w1", "m2b1", "m2w2", "m2b2"]

_pmapped = None


def _get_pmapped():
    global _pmapped
    if _pmapped is None:
        _pmapped = jax.pmap(_block, in_axes=(0, 0) + (None,) * len(_PARAM_NAMES))
    return _pmapped


def kernel(**inputs):
    x = np.asarray(inputs["x"], np.float32)
    y = np.asarray(inputs["y"], np.float32)
    params = [np.asarray(inputs[nm], np.float32) for nm in _PARAM_NAMES]
    per = B // NCORES
    xs = x.reshape(NCORES, per, N, C)
    ys = y.reshape(NCORES, per, N, N, C)
    fn = _get_pmapped()
    x_out, y_out, message = fn(xs, ys, *params)
    x_out = np.asarray(x_out).reshape(B, N, C)
    y_out = np.asarray(y_out).reshape(B, N, N, C)
    message = np.asarray(message).reshape(B, H, N, N)
    return x_out, y_out, message
